# revision 11
# baseline (speedup 1.0000x reference)
import os
import sys

for _p in ("/opt/trn_rl_repo", "/root/.axon_site/_ro/trn_rl_repo"):
    if os.path.isdir(_p) and _p not in sys.path:
        sys.path.insert(0, _p)

import numpy as np
from contextlib import ExitStack

import concourse.bass as bass
import concourse.tile as tile
from concourse import mybir
from concourse.bass_utils import run_bass_kernel_spmd
from concourse.masks import make_identity

F32 = mybir.dt.float32
AF = mybir.ActivationFunctionType
OP = mybir.AluOpType

B, C, H, W = 64, 3, 224, 224
PH = PW = 16
GH, GW = H // PH, W // PW
NP = GH * GW            # 196
E = 512
NH = 8
HD = E // NH            # 64
PPD = C * PH * PW       # 768
NM = 147                # masked
NU = 49                 # unmasked
NCORES = 8
BS = B // NCORES        # 8 samples per core

# token tiles (196 = 2 x 98, aligned to gh boundary: 7 gh rows each)
TT = [(0, 98), (98, 98)]
# masked-token tiles (147 = 128 + 19)
JT = [(0, 128), (128, 19)]
# 768-wide output chunks for PSUM (<=512 f32)
NC2 = [(0, 384), (384, 384)]

# pixel permutation: device free order is (c, ph, pw); reference is (ph, pw, c)
_i = np.arange(PPD)
_c, _ph, _pw = _i // 256, (_i % 256) // 16, _i % 16
CP2REF = (_ph * (PW * C) + _pw * C + _c).astype(np.int64)

_CACHE = {}


def _build_nc():
    nc = bass.Bass()
    dt = {}

    def din(name, shape):
        dt[name] = nc.dram_tensor(name, list(shape), F32, kind="ExternalInput")

    def dout(name, shape):
        dt[name] = nc.dram_tensor(name, list(shape), F32, kind="ExternalOutput")

    din("x", (BS, C, H, W))
    din("pos", (NP, E))
    din("wpeT", (PPD, E))
    din("bpe", (1, E))
    din("encWT", (E, E))
    din("encb", (1, E))
    din("wqT", (E, E))
    din("bq", (1, E))
    din("wkT", (E, E))
    din("bk", (1, E))
    din("wvT", (E, E))
    din("bv", (1, E))
    din("outWT", (E, E))
    din("outb", (1, E))
    din("headWT", (E, PPD))
    din("headb", (1, PPD))
    din("noise", (BS, NM, PPD))
    din("S", (BS, NP, NM))
    din("U", (BS, NP, NU))
    din("ST", (BS, NM, NP))
    din("msk", (BS, NP, 1))
    din("omsk", (BS, NP, 1))
    dout("rec", (BS, C, H, W))
    dout("noi", (BS, C, H, W))
    dout("mim", (BS, C, H, W))
    dout("msum", (1, 1))

    PATCH_RE = "c (gh ph) (gw pw) -> c ph gh gw pw"

    with tile.TileContext(nc) as tc, ExitStack() as ctx:
        pw_ = ctx.enter_context(tc.tile_pool(name="pw", bufs=1))
        pio = ctx.enter_context(tc.tile_pool(name="pio", bufs=2))
        pwk = ctx.enter_context(tc.tile_pool(name="pwk", bufs=1))
        pout = ctx.enter_context(tc.tile_pool(name="pout", bufs=1))
        psA = ctx.enter_context(tc.tile_pool(name="psA", bufs=2, space="PSUM"))
        psB = ctx.enter_context(tc.tile_pool(name="psB", bufs=2, space="PSUM"))
        psO = ctx.enter_context(tc.tile_pool(name="psO", bufs=2, space="PSUM"))
        psM = ctx.enter_context(tc.tile_pool(name="psM", bufs=1, space="PSUM"))

        # ---- persistent weights / constants ----
        ident = pw_.tile([128, 128], F32)
        make_identity(nc, ident[:])
        ones_r = pw_.tile([1, E], F32)
        nc.gpsimd.memset(ones_r[:], 1.0)
        ones_c = pw_.tile([128, 1], F32)
        nc.gpsimd.memset(ones_c[:], 1.0)

        wpe = [pw_.tile([128, E], F32, tag=f"wpe{i}", name=f"wpe{i}") for i in range(6)]
        for i in range(6):
            nc.sync.dma_start(wpe[i][:], dt["wpeT"][128 * i:128 * (i + 1), :])
        encW = [pw_.tile([128, E], F32, tag=f"encW{i}", name=f"encW{i}") for i in range(4)]
        wq = [pw_.tile([128, E], F32, tag=f"wq{i}", name=f"wq{i}") for i in range(4)]
        wk = [pw_.tile([128, E], F32, tag=f"wk{i}", name=f"wk{i}") for i in range(4)]
        wv = [pw_.tile([128, E], F32, tag=f"wv{i}", name=f"wv{i}") for i in range(4)]
        outW = [pw_.tile([128, E], F32, tag=f"outW{i}", name=f"outW{i}") for i in range(4)]
        headW = [pw_.tile([128, PPD], F32, tag=f"headW{i}", name=f"headW{i}") for i in range(4)]
        for i in range(4):
            nc.sync.dma_start(encW[i][:], dt["encWT"][128 * i:128 * (i + 1), :])
            nc.sync.dma_start(wq[i][:], dt["wqT"][128 * i:128 * (i + 1), :])
            nc.sync.dma_start(wk[i][:], dt["wkT"][128 * i:128 * (i + 1), :])
            nc.sync.dma_start(wv[i][:], dt["wvT"][128 * i:128 * (i + 1), :])
            nc.sync.dma_start(outW[i][:], dt["outWT"][128 * i:128 * (i + 1), :])
            nc.sync.dma_start(headW[i][:], dt["headWT"][128 * i:128 * (i + 1), :])
        pos_t = [pw_.tile([98, E], F32, tag=f"pos{i}", name=f"pos{i}") for i in range(2)]
        for i, (t0, tn) in enumerate(TT):
            nc.sync.dma_start(pos_t[i][:], dt["pos"][t0:t0 + tn, :])
        bias = {}
        for nm in ("bpe", "encb", "bq", "bk", "bv", "outb"):
            bias[nm] = pw_.tile([1, E], F32, tag=nm, name=nm)
            nc.sync.dma_start(bias[nm][:], dt[nm][:, :])
        bias["headb"] = pw_.tile([1, PPD], F32, tag="headb", name="headb")
        nc.sync.dma_start(bias["headb"][:], dt["headb"][:, :])

        pmse = psM.tile([1, 1], F32)

        for s in range(BS):
            first = s == 0
            last = s == BS - 1
            xsr = dt["x"][s].rearrange(PATCH_RE, ph=PH, pw=PW)

            # ---- patch load (N-layout, 2 x [98, 768]) ----
            pN = []
            for i, (t0, tn) in enumerate(TT):
                traw = pio.tile([98, PPD], F32, tag=f"pNraw{i}", name=f"pNraw{i}")
                eng = nc.sync if i == 0 else nc.scalar
                for c in range(C):
                    for ph in range(PH):
                        col = c * 256 + ph * 16
                        eng.dma_start(traw[:, col:col + 16],
                                      xsr[c, ph][7 * i:7 * (i + 1)])
                t = pio.tile([98, PPD], F32, tag=f"pN{i}", name=f"pN{i}")
                for g in range(24):
                    nc.vector.tensor_copy(t[:, 32 * g:32 * (g + 1)],
                                          traw[:, 32 * g:32 * (g + 1)])
                pN.append(t)
            Ss, Us, STs, nr = [], [], [], []
            for i, (t0, tn) in enumerate(TT):
                t = pio.tile([98, NM], F32, tag=f"S{i}", name=f"S{i}")
                nc.sync.dma_start(t[:], dt["S"][s, t0:t0 + tn, :])
                Ss.append(t)
                t = pio.tile([98, NU], F32, tag=f"U{i}", name=f"U{i}")
                nc.sync.dma_start(t[:], dt["U"][s, t0:t0 + tn, :])
                Us.append(t)
            for i, (j0, jn) in enumerate(JT):
                t = pio.tile([128, NP], F32, tag=f"ST{i}", name=f"ST{i}")
                nc.sync.dma_start(t[:jn, :], dt["ST"][s, j0:j0 + jn, :])
                STs.append(t)
                t = pio.tile([128, PPD], F32, tag=f"nr{i}", name=f"nr{i}")
                nc.sync.dma_start(t[:jn, :], dt["noise"][s, j0:j0 + jn, :])
                nr.append(t)
            mskT, omskT = [], []
            for i, (t0, tn) in enumerate(TT):
                t = pio.tile([98, 1], F32, tag=f"msk{i}", name=f"msk{i}")
                nc.sync.dma_start(t[:], dt["msk"][s, t0:t0 + tn, :])
                mskT.append(t)
                t = pio.tile([98, 1], F32, tag=f"omsk{i}", name=f"omsk{i}")
                nc.sync.dma_start(t[:], dt["omsk"][s, t0:t0 + tn, :])
                omskT.append(t)

            # ---- X_T = transpose(patches)  (6 x [128, 196]) ----
            xt = []
            for ft in range(6):
                ps = psA.tile([128, NP], F32, tag="psA")
                for i, (t0, tn) in enumerate(TT):
                    nc.tensor.transpose(
                        ps[:, t0:t0 + tn],
                        pN[i][:, 128 * ft:128 * (ft + 1)],
                        ident[:98, :98],
                    )
                t = pwk.tile([128, NP], F32, tag=f"xt{ft}", name=f"xt{ft}")
                nc.any.tensor_copy(t[:], ps[:])
                xt.append(t)

            # ---- tok_N = patches @ WpeT + bpe + pos  (2 x [98, 512]) ----
            tokn = []
            for i, (t0, tn) in enumerate(TT):
                ps = psB.tile([98, E], F32, tag="psB")
                for ft in range(6):
                    nc.tensor.matmul(
                        ps[:], lhsT=xt[ft][:, t0:t0 + tn], rhs=wpe[ft][:],
                        start=(ft == 0), stop=False,
                    )
                nc.tensor.matmul(
                    ps[:], lhsT=ones_r[:, :tn], rhs=bias["bpe"][:],
                    start=False, stop=True,
                )
                t = pwk.tile([98, E], F32, tag=f"tokn{i}", name=f"tokn{i}")
                nc.vector.tensor_tensor(t[:], ps[:], pos_t[i][:], op=OP.add)
                tokn.append(t)

            # ---- gather masked/unmasked tokens into T-layout ----
            mtok, utok = [], []
            for et in range(4):
                es = slice(128 * et, 128 * (et + 1))
                ps = psA.tile([128, NM], F32, tag="psA")
                for i in range(2):
                    nc.tensor.matmul(ps[:], lhsT=tokn[i][:, es], rhs=Ss[i][:],
                                     start=(i == 0), stop=(i == 1))
                t = pwk.tile([128, NM], F32, tag=f"mtok{et}", name=f"mtok{et}")
                nc.any.tensor_copy(t[:], ps[:])
                mtok.append(t)
                ps = psA.tile([128, NU], F32, tag="psA")
                for i in range(2):
                    nc.tensor.matmul(ps[:], lhsT=tokn[i][:, es], rhs=Us[i][:],
                                     start=(i == 0), stop=(i == 1))
                t = pwk.tile([128, NU], F32, tag=f"utok{et}", name=f"utok{et}")
                nc.any.tensor_copy(t[:], ps[:])
                utok.append(t)

            # ---- encoded = gelu(enc_W @ utok + encb)  (T-layout) ----
            enc = []
            for et in range(4):
                es = slice(128 * et, 128 * (et + 1))
                ps = psA.tile([128, NU], F32, tag="psA")
                for kt in range(4):
                    nc.tensor.matmul(ps[:], lhsT=encW[kt][:, es], rhs=utok[kt][:],
                                     start=(kt == 0), stop=False)
                nc.tensor.matmul(ps[:], lhsT=bias["encb"][:, es], rhs=ones_r[:, :NU],
                                 start=False, stop=True)
                t = pwk.tile([128, NU], F32, tag=f"enc{et}", name=f"enc{et}")
                nc.scalar.activation(t[:], ps[:], AF.Gelu_apprx_tanh)
                enc.append(t)

            # ---- q (from mtok), k/v (from enc) in T-layout ----
            def linT(wts, bnm, rhs_tiles, n, tagp):
                outs = []
                for et in range(4):
                    es = slice(128 * et, 128 * (et + 1))
                    ps = psA.tile([128, n], F32, tag="psA")
                    for kt in range(4):
                        nc.tensor.matmul(ps[:], lhsT=wts[kt][:, es],
                                         rhs=rhs_tiles[kt][:],
                                         start=(kt == 0), stop=False)
                    nc.tensor.matmul(ps[:], lhsT=bias[bnm][:, es],
                                     rhs=ones_r[:, :n], start=False, stop=True)
                    t = pwk.tile([128, n], F32, tag=f"{tagp}{et}", name=f"{tagp}{et}")
                    nc.any.tensor_copy(t[:], ps[:])
                    outs.append(t)
                return outs

            qT = linT(wq, "bq", mtok, NM, "qT")
            kT = linT(wk, "bk", enc, NU, "kT")
            vT = linT(wv, "bv", enc, NU, "vT")

            # v to N-layout [49, 512]
            vn = pwk.tile([NU, E], F32, tag="vn", name="vn")
            for et in range(4):
                ps = psA.tile([NU, 128], F32, tag="psA")
                nc.tensor.transpose(ps[:], vT[et][:], ident[:])
                nc.any.tensor_copy(vn[:, 128 * et:128 * (et + 1)], ps[:])

            # ---- attention (per head), accumulate attn-out in T-layout ----
            aT = []
            for pair in range(4):
                pao = psO.tile([128, NM], F32, tag="pao")
                for hh in range(2):
                    h = 2 * pair + hh
                    off = 64 * hh
                    psl = psA.tile([NU, NM], F32, tag="psA")
                    nc.tensor.matmul(psl[:],
                                     lhsT=kT[pair][off:off + 64, :],
                                     rhs=qT[pair][off:off + 64, :],
                                     start=True, stop=True)
                    Eh = pwk.tile([NU, NM], F32, tag="Eh", name="Eh")
                    nc.scalar.activation(Eh[:], psl[:], AF.Exp, scale=0.125)
                    pss = psB.tile([1, NM], F32, tag="psB")
                    nc.tensor.matmul(pss[:], lhsT=ones_c[:NU, :], rhs=Eh[:],
                                     start=True, stop=True)
                    rh = pwk.tile([1, NM], F32, tag="rh", name="rh")
                    nc.vector.reciprocal(rh[:], pss[:])
                    prb = psA.tile([NU, NM], F32, tag="psA")
                    nc.tensor.matmul(prb[:], lhsT=ones_r[:, :NU], rhs=rh[:],
                                     start=True, stop=True)
                    Ah = pwk.tile([NU, NM], F32, tag="Ah", name="Ah")
                    nc.vector.tensor_tensor(Ah[:], Eh[:], prb[:], op=OP.mult)
                    nc.tensor.matmul(pao[off:off + 64, :],
                                     lhsT=vn[:, 64 * h:64 * (h + 1)],
                                     rhs=Ah[:], start=True, stop=True)
                t = pwk.tile([128, NM], F32, tag=f"aT{pair}", name=f"aT{pair}")
                nc.any.tensor_copy(t[:], pao[:])
                aT.append(t)

            # ---- out-proj (T-layout) ----
            pT = linT(outW, "outb", aT, NM, "pT")

            # ---- pred (N-layout, j-tiles) + masked-patch gather ----
            pn = [pwk.tile([128, PPD], F32, tag=f"pn{i}", name=f"pn{i}") for i in range(2)]
            mp = [pwk.tile([128, PPD], F32, tag=f"mp{i}", name=f"mp{i}") for i in range(2)]
            for ji, (j0, jn) in enumerate(JT):
                for ci, (n0, nn) in enumerate(NC2):
                    ps = psB.tile([128, 384], F32, tag="psB")
                    for kt in range(4):
                        nc.tensor.matmul(ps[:jn, :],
                                         lhsT=pT[kt][:, j0:j0 + jn],
                                         rhs=headW[kt][:, n0:n0 + nn],
                                         start=(kt == 0), stop=False)
                    nc.tensor.matmul(ps[:jn, :], lhsT=ones_r[:, :jn],
                                     rhs=bias["headb"][:, n0:n0 + nn],
                                     start=False, stop=True)
                    nc.any.tensor_copy(pn[ji][:jn, n0:n0 + nn], ps[:jn, :])
                    ps = psB.tile([128, 384], F32, tag="psB")
                    for i in range(2):
                        nc.tensor.matmul(ps[:jn, :],
                                         lhsT=Ss[i][:, j0:j0 + jn],
                                         rhs=pN[i][:, n0:n0 + nn],
                                         start=(i == 0), stop=(i == 1))
                    nc.any.tensor_copy(mp[ji][:jn, n0:n0 + nn], ps[:jn, :])

            # ---- deltas, mse ----
            dl = [pwk.tile([128, PPD], F32, tag=f"dl{i}", name=f"dl{i}") for i in range(2)]
            dl2 = [pwk.tile([128, PPD], F32, tag=f"dl2{i}", name=f"dl2{i}") for i in range(2)]
            for ji, (j0, jn) in enumerate(JT):
                nc.vector.tensor_tensor(dl[ji][:jn, :], pn[ji][:jn, :],
                                        mp[ji][:jn, :], op=OP.subtract)
                nrp = nr[ji][:jn, :].rearrange("p (ph pw c) -> p c ph pw",
                                               ph=PH, pw=PW, c=C)
                d2v = dl2[ji][:jn, :].rearrange("p (c ph pw) -> p c ph pw",
                                                ph=PH, pw=PW, c=C)
                mpv = mp[ji][:jn, :].rearrange("p (c ph pw) -> p c ph pw",
                                               ph=PH, pw=PW, c=C)
                nc.vector.tensor_tensor(d2v, nrp, mpv, op=OP.subtract)
                ab = pwk.tile([128, 1], F32, tag=f"ab{ji}", name=f"ab{ji}")
                nc.vector.tensor_reduce(ab[:jn, :], dl[ji][:jn, :],
                                        axis=mybir.AxisListType.X, op=OP.add,
                                        apply_absolute_value=True)
                nc.tensor.matmul(pmse[:], lhsT=ab[:jn, :], rhs=ones_c[:jn, :],
                                 start=(first and ji == 0), stop=(last and ji == 1),
                                 skip_group_check=True)

            # ---- scatter recons / noisy, mask-image; write outputs ----
            for i, (t0, tn) in enumerate(TT):
                recn = pout.tile([98, PPD], F32, tag=f"recn{i}", name=f"recn{i}")
                noin = pout.tile([98, PPD], F32, tag=f"noin{i}", name=f"noin{i}")
                for ci, (n0, nn) in enumerate(NC2):
                    for src, dst in ((dl, recn), (dl2, noin)):
                        ps = psB.tile([128, 384], F32, tag="psB")
                        for ji, (j0, jn) in enumerate(JT):
                            nc.tensor.matmul(ps[:tn, :],
                                             lhsT=STs[ji][:jn, t0:t0 + tn],
                                             rhs=src[ji][:jn, n0:n0 + nn],
                                             start=(ji == 0), stop=(ji == 1))
                        nc.vector.tensor_tensor(dst[:, n0:n0 + nn], ps[:tn, :],
                                                pN[i][:, n0:n0 + nn], op=OP.add)
                mi = pout.tile([98, PPD], F32, tag=f"mi{i}", name=f"mi{i}")
                nc.vector.tensor_scalar(mi[:], pN[i][:], omskT[i][:], mskT[i][:],
                                        op0=OP.mult, op1=OP.add)
                for k, (nm, t) in enumerate((("rec", recn), ("noi", noin),
                                             ("mim", mi))):
                    orr = dt[nm][s].rearrange(PATCH_RE, ph=PH, pw=PW)
                    for c in range(C):
                        for ph in range(PH):
                            col = c * 256 + ph * 16
                            eng = (nc.sync if (c * PH + ph + k) % 2 == 0
                                   else nc.scalar)
                            eng.dma_start(orr[c, ph][7 * i:7 * (i + 1)],
                                          t[:, col:col + 16])

        # ---- mse out ----
        msb = pw_.tile([1, 1], F32, tag="msb")
        nc.any.tensor_copy(msb[:], pmse[:])
        nc.sync.dma_start(dt["msum"][:, :], msb[:])

    from concourse.mybir import _bass_rust
    try:
        _bass_rust.move_matmul_waits_to_ldweights(nc.m)
    except Exception:
        pass
    _bass_rust.generate_event_semaphores(nc)
    return nc


def _host_prep(x, shuffle_indices, W_pe, b_pe, pos_embed, enc_W, enc_b,
               in_proj_w, in_proj_b, out_w, out_b, head_W, head_b, noise):
    f32 = np.float32
    Wq, Wk, Wv = (np.ascontiguousarray(a) for a in np.split(in_proj_w, 3))
    bq, bk, bv = np.split(in_proj_b, 3)
    shared = {
        "pos": np.ascontiguousarray(pos_embed[0], f32),
        "wpeT": np.ascontiguousarray(W_pe.T[CP2REF], f32),
        "bpe": np.ascontiguousarray(b_pe[None, :], f32),
        "encWT": np.ascontiguousarray(enc_W.T, f32),
        "encb": np.ascontiguousarray(enc_b[None, :], f32),
        "wqT": np.ascontiguousarray(Wq.T, f32),
        "bq": np.ascontiguousarray(bq[None, :], f32),
        "wkT": np.ascontiguousarray(Wk.T, f32),
        "bk": np.ascontiguousarray(bk[None, :], f32),
        "wvT": np.ascontiguousarray(Wv.T, f32),
        "bv": np.ascontiguousarray(bv[None, :], f32),
        "outWT": np.ascontiguousarray(out_w.T, f32),
        "outb": np.ascontiguousarray(out_b[None, :], f32),
        "headWT": np.ascontiguousarray(head_W.T[:, CP2REF], f32),
        "headb": np.ascontiguousarray(head_b[CP2REF][None, :], f32),
    }
    in_maps = []
    bi = np.arange(BS)[:, None]
    for c in range(NCORES):
        sl = slice(c * BS, (c + 1) * BS)
        sh = shuffle_indices[sl]
        mi_, ui_ = sh[:, :NM], sh[:, NM:]
        S = np.zeros((BS, NP, NM), f32)
        S[bi, mi_, np.arange(NM)[None, :]] = 1.0
        U = np.zeros((BS, NP, NU), f32)
        U[bi, ui_, np.arange(NU)[None, :]] = 1.0
        msk = np.minimum(S.sum(-1), 1.0)[:, :, None]
        m = dict(shared)
        m["x"] = np.ascontiguousarray(x[sl], f32)
        m["noise"] = np.ascontiguousarray(noise[sl], f32)
        m["S"] = S
        m["U"] = U
        m["ST"] = np.ascontiguousarray(S.transpose(0, 2, 1))
        m["msk"] = np.ascontiguousarray(msk)
        m["omsk"] = np.ascontiguousarray(1.0 - msk)
        in_maps.append(m)
    return in_maps


def kernel(**inputs):
    if "nc" not in _CACHE:
        _CACHE["nc"] = _build_nc()
    nc = _CACHE["nc"]
    in_maps = _host_prep(**inputs)
    res = run_bass_kernel_spmd(nc, in_maps, list(range(NCORES))).results
    rec = np.concatenate([r["rec"] for r in res], axis=0)
    noi = np.concatenate([r["noi"] for r in res], axis=0)
    mim = np.concatenate([r["mim"] for r in res], axis=0)
    mse = np.float32(sum(float(r["msum"][0, 0]) for r in res) / (B * NM * PPD))
    return rec, noi, mse, mim
